# revision 3
# baseline (speedup 1.0000x reference)
"""Trainium2 Bass kernel for nn_DecoderModel_42228118454332.

Key algebraic structure of the reference model:
  - The 4-layer alignment MLP has no nonlinearities, so it composes into a
    single affine map e = x . m + c with m = W1^T W2^T W3^T W4^T.
  - x = [S | padded]; the S-dependent part of e is constant over encoder
    positions t, so it cancels inside softmax(axis=t). Attention weights
    therefore do not depend on the decoder state S at all.
  - The LSTM starts from zero state each step (w_hh sees h0=c0=0) and its
    input (the context) is step-invariant, so the output h is identical for
    all 50 decoder steps. The f-gate multiplies c0=0 and is never used.

Device computation per core k (SPMD over 8 cores):
  - compose m_P = W1P^T @ (W2^T @ (W3^T @ W4^T)) on PE (only the `padded`
    1024:3072 slice of the input features matters).
  - batch shard: core k owns batches [8k, 8k+8). padT = padded^T [2048, 400]
    feat-major. e = m_P . padT on PE -> [1, (b,t)]; softmax over t on
    partition 0; broadcast a to 128 partitions; context^T [2048, 8] via DVE
    multiply+reduce.
  - AllGather context -> [16384, 8] (8 ranks x 2048 feat x 8 batches).
  - LSTM tensor-sharded over the hidden dim: core k owns h rows
    [128k, 128(k+1)) of every layer, i.e. the matching i/g/o gate rows.
    gates = W_sel^T.T @ ct^T on PE; sigmoid/tanh on ACT; h slice [128, 64];
    AllGather h^T between layers. Final layer writes the h^T slice out.
Host: concat slices -> h^T [1024, 64] -> h [64, 1024] -> broadcast to
  [50, 64, 1024].
"""

import sys

for _p in ("/opt/trn_rl_repo", "/root/.axon_site/_ro/trn_rl_repo"):
    if _p not in sys.path:
        sys.path.insert(0, _p)

import numpy as np

from concourse import bacc, mybir, tile
from concourse.bass_utils import run_bass_kernel_spmd

H = 1024          # hidden size
HH = 2 * H        # encoder feature size
T = 50            # encoder length == decoder steps
B = 64            # batch
AH = 256          # alignment hidden
NC = 8            # cores
BL = B // NC      # batches per core (8)
BT = BL * T       # 400
FO = HH // 128    # 16 feature chunks of padded part
KT1 = H // 128    # 8 k-tiles for layers 1..3

F32 = mybir.dt.float32
RG = [list(range(NC))]

_CACHE = {}


def _build():
    nc = bacc.Bacc("TRN2", target_bir_lowering=False, debug=False, num_devices=NC)

    # ---- kernel I/O ----
    padT = nc.dram_tensor("padT", [HH, BT], F32, kind="ExternalInput")
    w1p = nc.dram_tensor("w1p", [AH, HH], F32, kind="ExternalInput")
    w2 = nc.dram_tensor("w2", [AH, AH], F32, kind="ExternalInput")
    w3 = nc.dram_tensor("w3", [AH, AH], F32, kind="ExternalInput")
    w4t = nc.dram_tensor("w4t", [AH, 1], F32, kind="ExternalInput")
    wl = [
        nc.dram_tensor(f"wl{l}", [HH if l == 0 else H, 384], F32, kind="ExternalInput")
        for l in range(4)
    ]
    bih = [nc.dram_tensor(f"bih{l}", [128, 3], F32, kind="ExternalInput") for l in range(4)]
    bhh = [nc.dram_tensor(f"bhh{l}", [128, 3], F32, kind="ExternalInput") for l in range(4)]
    out = nc.dram_tensor("out", [128, B], F32, kind="ExternalOutput")

    # ---- collective internals ----
    ct_in = nc.dram_tensor("ct_in", [HH, BL], F32)
    ct_ag = nc.dram_tensor("ct_ag", [HH * NC, BL], F32, addr_space="Shared")
    h_in = [nc.dram_tensor(f"h_in{l}", [128, B], F32) for l in range(3)]
    h_ag = [nc.dram_tensor(f"h_ag{l}", [H, B], F32, addr_space="Shared") for l in range(3)]

    with tile.TileContext(nc) as tc:
        with (
            tc.tile_pool(name="big", bufs=1) as big,
            tc.tile_pool(name="work", bufs=3) as work,
            tc.tile_pool(name="psum", bufs=1, space="PSUM") as pp,
            tc.tile_pool(name="psg", bufs=4, space="PSUM") as pg,
        ):
            # ---------- DMA loads ----------
            # padded^T, chunked so e-matmuls can start before the full load
            padT_sb = big.tile([128, FO, BT], F32, tag="padT_sb")
            N_PCHUNK = 4
            for c in range(N_PCHUNK):
                fo0 = c * (FO // N_PCHUNK)
                fo1 = (c + 1) * (FO // N_PCHUNK)
                nc.sync.dma_start(
                    padT_sb[:, fo0:fo1, :],
                    padT.ap()[fo0 * 128 : fo1 * 128, :].rearrange(
                        "(fo p) bt -> p fo bt", p=128
                    ),
                )

            w4t_sb = big.tile([128, 2, 1], F32, tag="w4t_sb")
            nc.sync.dma_start(w4t_sb[:], w4t.ap().rearrange("(kt p) o -> p kt o", p=128))
            w3_sb = big.tile([128, 2, AH], F32, tag="w3_sb")
            nc.sync.dma_start(w3_sb[:], w3.ap().rearrange("(kt p) j -> p kt j", p=128))
            w2_sb = big.tile([128, 2, AH], F32, tag="w2_sb")
            nc.sync.dma_start(w2_sb[:], w2.ap().rearrange("(kt p) j -> p kt j", p=128))
            w1p_sb = big.tile([128, 2, HH], F32, tag="w1p_sb")
            nc.sync.dma_start(w1p_sb[:], w1p.ap().rearrange("(kt p) j -> p kt j", p=128))

            wl_sb = []
            for l in range(4):
                kt = FO if l == 0 else KT1
                t_ = big.tile([128, kt, 384], F32, tag=f"wl{l}_sb")
                nc.sync.dma_start(
                    t_[:], wl[l].ap().rearrange("(kt p) j -> p kt j", p=128)
                )
                wl_sb.append(t_)

            b_sb = []
            for l in range(4):
                t_i = work.tile([128, 3], F32, tag=f"bih{l}")
                nc.sync.dma_start(t_i[:], bih[l].ap())
                t_h = work.tile([128, 3], F32, tag=f"bhh{l}")
                nc.sync.dma_start(t_h[:], bhh[l].ap())
                t_b = big.tile([128, 3], F32, tag=f"b{l}")
                nc.vector.tensor_add(t_b[:], t_i[:], t_h[:])
                b_sb.append(t_b)

            # ---------- compose m_P on PE ----------
            ps_u = pp.tile([128, 2], F32, tag="ps_u")
            for mt in range(2):
                for kt in range(2):
                    nc.tensor.matmul(
                        ps_u[:, mt : mt + 1],
                        w3_sb[:, kt, mt * 128 : (mt + 1) * 128],
                        w4t_sb[:, kt, :],
                        start=(kt == 0),
                        stop=(kt == 1),
                    )
            u1_sb = big.tile([128, 2], F32, tag="u1_sb")
            nc.vector.tensor_copy(u1_sb[:], ps_u[:])

            ps_u2 = pp.tile([128, 2], F32, tag="ps_u")
            for mt in range(2):
                for kt in range(2):
                    nc.tensor.matmul(
                        ps_u2[:, mt : mt + 1],
                        w2_sb[:, kt, mt * 128 : (mt + 1) * 128],
                        u1_sb[:, kt : kt + 1],
                        start=(kt == 0),
                        stop=(kt == 1),
                    )
            u2_sb = big.tile([128, 2], F32, tag="u2_sb")
            nc.vector.tensor_copy(u2_sb[:], ps_u2[:])

            ps_m = pp.tile([128, FO], F32, tag="ps_m")
            for mt in range(FO):
                for kt in range(2):
                    nc.tensor.matmul(
                        ps_m[:, mt : mt + 1],
                        w1p_sb[:, kt, mt * 128 : (mt + 1) * 128],
                        u2_sb[:, kt : kt + 1],
                        start=(kt == 0),
                        stop=(kt == 1),
                    )
            m_sb = big.tile([128, FO], F32, tag="m_sb")
            nc.vector.tensor_copy(m_sb[:], ps_m[:])

            # ---------- e = m_P . padT  -> [1, (b, t)] ----------
            e_ps = pp.tile([1, BT], F32, tag="e_ps")
            for fo in range(FO):
                nc.tensor.matmul(
                    e_ps[:],
                    m_sb[:, fo : fo + 1],
                    padT_sb[:, fo, :],
                    start=(fo == 0),
                    stop=(fo == FO - 1),
                )

            # ---------- softmax over t (partition 0) ----------
            e_sb = big.tile([1, BT], F32, tag="e_sb")
            nc.scalar.activation(e_sb[:], e_ps[:], mybir.ActivationFunctionType.Copy)
            e3 = e_sb[:].rearrange("p (b t) -> p b t", b=BL)
            mx = big.tile([1, BL], F32, tag="mx")
            nc.vector.reduce_max(mx[:], e3, axis=mybir.AxisListType.X)
            ec = big.tile([1, BT], F32, tag="ec")
            nc.vector.tensor_sub(
                ec[:].rearrange("p (b t) -> p b t", b=BL),
                e3,
                mx[:].unsqueeze(2).broadcast_to([1, BL, T]),
            )
            p_sb = big.tile([1, BT], F32, tag="p_sb")
            nc.scalar.activation(p_sb[:], ec[:], mybir.ActivationFunctionType.Exp)
            p3 = p_sb[:].rearrange("p (b t) -> p b t", b=BL)
            s_sb = big.tile([1, BL], F32, tag="s_sb")
            nc.vector.reduce_sum(s_sb[:], p3, axis=mybir.AxisListType.X)
            rs = big.tile([1, BL], F32, tag="rs")
            nc.vector.reciprocal(rs[:], s_sb[:])
            a_sb = big.tile([1, BT], F32, tag="a_sb")
            nc.vector.tensor_mul(
                a_sb[:].rearrange("p (b t) -> p b t", b=BL),
                p3,
                rs[:].unsqueeze(2).broadcast_to([1, BL, T]),
            )
            a_bc = big.tile([128, BT], F32, tag="a_bc")
            nc.gpsimd.partition_broadcast(a_bc[:], a_sb[:])

            # ---------- context^T via DVE ----------
            ctx_sb = big.tile([128, FO, BL], F32, tag="ctx_sb")
            for fo in range(FO):
                prod = work.tile([128, BT], F32, tag="prod")
                nc.vector.tensor_mul(prod[:], padT_sb[:, fo, :], a_bc[:])
                nc.vector.tensor_reduce(
                    ctx_sb[:, fo, :],
                    prod[:].rearrange("p (b t) -> p b t", b=BL),
                    op=mybir.AluOpType.add,
                    axis=mybir.AxisListType.X,
                )

            nc.sync.dma_start(
                ct_in.ap().rearrange("(fo p) b -> p fo b", p=128), ctx_sb[:]
            )
            nc.gpsimd.collective_compute(
                "AllGather",
                mybir.AluOpType.bypass,
                replica_groups=RG,
                ins=[ct_in.ap()],
                outs=[ct_ag.ap()],
            )
            ctT_sb = big.tile([128, FO, NC, BL], F32, tag="ctT_sb")
            ct_ag_r = ct_ag.ap().rearrange("(j fo p) b -> fo p j b", j=NC, fo=FO, p=128)
            for fo in range(FO):
                nc.sync.dma_start(ctT_sb[:, fo, :, :], ct_ag_r[fo])

            # ---------- 4-layer LSTM cell (i, g, o gates only) ----------
            rhs_sb = ctT_sb  # [128, kt, ...] with free size 64 per kt
            for l in range(4):
                kt_n = FO if l == 0 else KT1
                gates = []
                for m in range(3):
                    ps_g = pg.tile([128, B], F32, tag="gates")
                    for kt in range(kt_n):
                        if l == 0:
                            rhs = rhs_sb[:, kt, :, :]
                        else:
                            rhs = rhs_sb[:, kt, :]
                        nc.tensor.matmul(
                            ps_g[:],
                            wl_sb[l][:, kt, m * 128 : (m + 1) * 128],
                            rhs,
                            start=(kt == 0),
                            stop=(kt == kt_n - 1),
                        )
                    gates.append(ps_g)
                sig_i = work.tile([128, B], F32, tag="sig_i")
                nc.scalar.activation(
                    sig_i[:], gates[0][:], mybir.ActivationFunctionType.Sigmoid,
                    bias=b_sb[l][:, 0:1],
                )
                tanh_g = work.tile([128, B], F32, tag="tanh_g")
                nc.scalar.activation(
                    tanh_g[:], gates[1][:], mybir.ActivationFunctionType.Tanh,
                    bias=b_sb[l][:, 1:2],
                )
                c_t = work.tile([128, B], F32, tag="c_t")
                nc.vector.tensor_mul(c_t[:], sig_i[:], tanh_g[:])
                tanh_c = work.tile([128, B], F32, tag="tanh_c")
                nc.scalar.activation(
                    tanh_c[:], c_t[:], mybir.ActivationFunctionType.Tanh
                )
                sig_o = work.tile([128, B], F32, tag="sig_o")
                nc.scalar.activation(
                    sig_o[:], gates[2][:], mybir.ActivationFunctionType.Sigmoid,
                    bias=b_sb[l][:, 2:3],
                )
                h_sl = work.tile([128, B], F32, tag="h_sl")
                nc.vector.tensor_mul(h_sl[:], sig_o[:], tanh_c[:])

                if l < 3:
                    nc.sync.dma_start(h_in[l].ap(), h_sl[:])
                    nc.gpsimd.collective_compute(
                        "AllGather",
                        mybir.AluOpType.bypass,
                        replica_groups=RG,
                        ins=[h_in[l].ap()],
                        outs=[h_ag[l].ap()],
                    )
                    nxt = big.tile([128, KT1, B], F32, tag=f"h{l}_sb")
                    nc.sync.dma_start(
                        nxt[:], h_ag[l].ap().rearrange("(kt p) b -> p kt b", p=128)
                    )
                    rhs_sb = nxt
                else:
                    nc.sync.dma_start(out.ap(), h_sl[:])

    nc.compile()
    return nc


def _prep_inputs(padded, align_ws, w_ih, b_ih, b_hh):
    """Build the 8 per-core input maps (host-side sharding / layout only)."""
    padded = np.asarray(padded, dtype=np.float32)
    w1 = np.asarray(align_ws[0], dtype=np.float32)
    shared = {
        "w1p": np.ascontiguousarray(w1[:, H:]),          # [256, 2048]
        "w2": np.ascontiguousarray(np.asarray(align_ws[1], dtype=np.float32)),
        "w3": np.ascontiguousarray(np.asarray(align_ws[2], dtype=np.float32)),
        "w4t": np.ascontiguousarray(
            np.asarray(align_ws[3], dtype=np.float32).reshape(AH, 1)
        ),
    }
    in_maps = []
    for k in range(NC):
        m = dict(shared)
        # padded^T for batches [8k, 8k+8): [2048, 8, 50] -> [2048, 400]
        sl = padded[:, k * BL : (k + 1) * BL, :]          # [50, 8, 2048]
        m["padT"] = np.ascontiguousarray(sl.transpose(2, 1, 0)).reshape(HH, BT)
        for l in range(4):
            w = np.asarray(w_ih[l], dtype=np.float32)
            rows = np.concatenate(
                [w[g * H + k * 128 : g * H + (k + 1) * 128, :] for g in (0, 2, 3)],
                axis=0,
            )                                              # [384, fin] i,g,o rows
            m[f"wl{l}"] = np.ascontiguousarray(rows.T)     # [fin, 384]
            for name, b in (("bih", b_ih[l]), ("bhh", b_hh[l])):
                bb = np.asarray(b, dtype=np.float32)
                bsel = np.stack(
                    [bb[g * H + k * 128 : g * H + (k + 1) * 128] for g in (0, 2, 3)],
                    axis=1,
                )                                          # [128, 3]
                m[f"{name}{l}"] = np.ascontiguousarray(bsel)
        in_maps.append(m)
    return in_maps


def kernel(padded, align_ws, align_bs, w_ih, w_hh, b_ih, b_hh):
    # align_bs and w_hh are mathematically inert: the MLP biases add a
    # t-invariant constant to e (cancels in softmax); w_hh multiplies the
    # zero initial LSTM state.
    if "nc" not in _CACHE:
        _CACHE["nc"] = _build()
    nc = _CACHE["nc"]
    in_maps = _prep_inputs(padded, align_ws, w_ih, b_ih, b_hh)
    res = run_bass_kernel_spmd(nc, in_maps, list(range(NC)))
    hT = np.concatenate([res.results[k]["out"] for k in range(NC)], axis=0)  # [1024, 64]
    h = hT.T                                                                  # [64, 1024]
    return np.ascontiguousarray(
        np.broadcast_to(h[None, :, :], (T, B, H))
    ).astype(np.float32)


if __name__ == "__main__":
    rng = np.random.default_rng(0)
    pass


# revision 4
# speedup vs baseline: 1.1798x; 1.1798x over previous
"""Trainium2 Bass kernel for nn_DecoderModel_42228118454332.

Key algebraic structure of the reference model:
  - The 4-layer alignment MLP has no nonlinearities, so it composes into a
    single affine map e = x . m + c with m = W1^T W2^T W3^T W4^T.
  - x = [S | padded]; the S-dependent part of e is constant over encoder
    positions t, so it cancels inside softmax(axis=t). Attention weights
    therefore do not depend on the decoder state S at all.
  - The LSTM starts from zero state each step (w_hh sees h0=c0=0) and its
    input (the context) is step-invariant, so the output h is identical for
    all 50 decoder steps. The f-gate multiplies c0=0 and is never used.

Device computation per core k (SPMD over 8 cores):
  - compose m_P = W1P^T @ (W2^T @ (W3^T @ W4^T)) on PE (only the `padded`
    1024:3072 slice of the input features matters).
  - batch shard: core k owns batches [8k, 8k+8). padT = padded^T [2048, 400]
    feat-major. e = m_P . padT on PE -> [1, (b,t)]; softmax over t on
    partition 0; broadcast a to 128 partitions; context^T [2048, 8] via DVE
    multiply+reduce in fp32.
  - AllGather context (bf16) -> [16384, 8].
  - LSTM tensor-sharded over the hidden dim: core k owns h rows
    [128k, 128(k+1)) of every layer, i.e. the matching i/g/o gate rows.
    gates = W_sel^T.T @ ct^T on PE (bf16 in, fp32 accum); sigmoid/tanh on
    ACT in fp32; AllGather h^T (bf16) between layers. Final layer writes
    the fp32 h^T slice out.
Host: concat slices -> h^T [1024, 64] -> h [64, 1024] -> broadcast to
  [50, 64, 1024].

Matmul operands are bf16: fp32 matmuls on TRN2 lower to HI/LO pairs (2x
instructions, ~4x PE time) and forgo fast weight load.
"""

import sys

for _p in ("/opt/trn_rl_repo", "/root/.axon_site/_ro/trn_rl_repo"):
    if _p not in sys.path:
        sys.path.insert(0, _p)

import ml_dtypes
import numpy as np

from concourse import bacc, mybir, tile
from concourse.bass_utils import run_bass_kernel_spmd

H = 1024          # hidden size
HH = 2 * H        # encoder feature size
T = 50            # encoder length == decoder steps
B = 64            # batch
AH = 256          # alignment hidden
NC = 8            # cores
BL = B // NC      # batches per core (8)
BT = BL * T       # 400
FO = HH // 128    # 16 feature chunks of padded part
KT1 = H // 128    # 8 k-tiles for layers 1..3

F32 = mybir.dt.float32
BF16 = mybir.dt.bfloat16
NPBF = ml_dtypes.bfloat16
RG = [list(range(NC))]

_CACHE = {}


def _build():
    nc = bacc.Bacc("TRN2", target_bir_lowering=False, debug=False, num_devices=NC)

    # ---- kernel I/O ----
    padT = nc.dram_tensor("padT", [HH, BT], F32, kind="ExternalInput")
    padTb = nc.dram_tensor("padTb", [HH, BT], BF16, kind="ExternalInput")
    w1p = nc.dram_tensor("w1p", [AH, HH], BF16, kind="ExternalInput")
    w2 = nc.dram_tensor("w2", [AH, AH], BF16, kind="ExternalInput")
    w3 = nc.dram_tensor("w3", [AH, AH], BF16, kind="ExternalInput")
    w4t = nc.dram_tensor("w4t", [AH, 1], BF16, kind="ExternalInput")
    wl = [
        nc.dram_tensor(f"wl{l}", [HH if l == 0 else H, 384], BF16, kind="ExternalInput")
        for l in range(4)
    ]
    bih = [nc.dram_tensor(f"bih{l}", [128, 3], F32, kind="ExternalInput") for l in range(4)]
    bhh = [nc.dram_tensor(f"bhh{l}", [128, 3], F32, kind="ExternalInput") for l in range(4)]
    out = nc.dram_tensor("out", [128, B], F32, kind="ExternalOutput")

    # ---- collective internals (bf16) ----
    ct_in = nc.dram_tensor("ct_in", [HH, BL], BF16)
    ct_ag = nc.dram_tensor("ct_ag", [HH * NC, BL], BF16, addr_space="Shared")
    h_in = [nc.dram_tensor(f"h_in{l}", [128, B], BF16) for l in range(3)]
    h_ag = [nc.dram_tensor(f"h_ag{l}", [H, B], BF16, addr_space="Shared") for l in range(3)]

    with tile.TileContext(nc) as tc:
        with (
            tc.tile_pool(name="big", bufs=1) as big,
            tc.tile_pool(name="work", bufs=3) as work,
            tc.tile_pool(name="psum", bufs=1, space="PSUM") as pp,
            tc.tile_pool(name="psg", bufs=4, space="PSUM") as pg,
        ):
            # ---------- DMA loads ----------
            # bf16 padded^T, chunked so e-matmuls can start early (sync ring)
            padTb_sb = big.tile([128, FO, BT], BF16, tag="padTb_sb")
            N_PCHUNK = 4
            for c in range(N_PCHUNK):
                fo0 = c * (FO // N_PCHUNK)
                fo1 = (c + 1) * (FO // N_PCHUNK)
                nc.sync.dma_start(
                    padTb_sb[:, fo0:fo1, :],
                    padTb.ap()[fo0 * 128 : fo1 * 128, :].rearrange(
                        "(fo p) bt -> p fo bt", p=128
                    ),
                )

            # fp32 padded^T for the context accumulation (scalar ring)
            padT_sb = big.tile([128, FO, BT], F32, tag="padT_sb")
            for c in range(N_PCHUNK):
                fo0 = c * (FO // N_PCHUNK)
                fo1 = (c + 1) * (FO // N_PCHUNK)
                nc.scalar.dma_start(
                    padT_sb[:, fo0:fo1, :],
                    padT.ap()[fo0 * 128 : fo1 * 128, :].rearrange(
                        "(fo p) bt -> p fo bt", p=128
                    ),
                )

            w4t_sb = big.tile([128, 2, 1], BF16, tag="w4t_sb")
            nc.sync.dma_start(w4t_sb[:], w4t.ap().rearrange("(kt p) o -> p kt o", p=128))
            w3_sb = big.tile([128, 2, AH], BF16, tag="w3_sb")
            nc.sync.dma_start(w3_sb[:], w3.ap().rearrange("(kt p) j -> p kt j", p=128))
            w2_sb = big.tile([128, 2, AH], BF16, tag="w2_sb")
            nc.sync.dma_start(w2_sb[:], w2.ap().rearrange("(kt p) j -> p kt j", p=128))
            w1p_sb = big.tile([128, 2, HH], BF16, tag="w1p_sb")
            nc.sync.dma_start(w1p_sb[:], w1p.ap().rearrange("(kt p) j -> p kt j", p=128))

            wl_sb = []
            for l in range(4):
                kt = FO if l == 0 else KT1
                t_ = big.tile([128, kt, 384], BF16, tag=f"wl{l}_sb")
                # weights go on the scalar HWDGE ring so they don't block
                # the latency-critical sync-ring DMAs
                nc.scalar.dma_start(
                    t_[:], wl[l].ap().rearrange("(kt p) j -> p kt j", p=128)
                )
                wl_sb.append(t_)

            b_sb = []
            for l in range(4):
                t_i = work.tile([128, 3], F32, tag=f"bih{l}")
                nc.sync.dma_start(t_i[:], bih[l].ap())
                t_h = work.tile([128, 3], F32, tag=f"bhh{l}")
                nc.sync.dma_start(t_h[:], bhh[l].ap())
                t_b = big.tile([128, 3], F32, tag=f"b{l}")
                nc.vector.tensor_add(t_b[:], t_i[:], t_h[:])
                b_sb.append(t_b)

            # ---------- compose m_P on PE (bf16 in, fp32 accum) ----------
            ps_u = pp.tile([128, 2], F32, tag="ps_u")
            for mt in range(2):
                for kt in range(2):
                    nc.tensor.matmul(
                        ps_u[:, mt : mt + 1],
                        w3_sb[:, kt, mt * 128 : (mt + 1) * 128],
                        w4t_sb[:, kt, :],
                        start=(kt == 0),
                        stop=(kt == 1),
                    )
            u1_sb = big.tile([128, 2], BF16, tag="u1_sb")
            nc.vector.tensor_copy(u1_sb[:], ps_u[:])

            ps_u2 = pp.tile([128, 2], F32, tag="ps_u")
            for mt in range(2):
                for kt in range(2):
                    nc.tensor.matmul(
                        ps_u2[:, mt : mt + 1],
                        w2_sb[:, kt, mt * 128 : (mt + 1) * 128],
                        u1_sb[:, kt : kt + 1],
                        start=(kt == 0),
                        stop=(kt == 1),
                    )
            u2_sb = big.tile([128, 2], BF16, tag="u2_sb")
            nc.vector.tensor_copy(u2_sb[:], ps_u2[:])

            ps_m = pp.tile([128, FO], F32, tag="ps_m")
            for mt in range(FO):
                for kt in range(2):
                    nc.tensor.matmul(
                        ps_m[:, mt : mt + 1],
                        w1p_sb[:, kt, mt * 128 : (mt + 1) * 128],
                        u2_sb[:, kt : kt + 1],
                        start=(kt == 0),
                        stop=(kt == 1),
                    )
            m_sb = big.tile([128, FO], BF16, tag="m_sb")
            nc.vector.tensor_copy(m_sb[:], ps_m[:])

            # ---------- e = m_P . padT  -> [1, (b, t)] ----------
            e_ps = pp.tile([1, BT], F32, tag="e_ps")
            for fo in range(FO):
                nc.tensor.matmul(
                    e_ps[:],
                    m_sb[:, fo : fo + 1],
                    padTb_sb[:, fo, :],
                    start=(fo == 0),
                    stop=(fo == FO - 1),
                )

            # ---------- softmax over t (partition 0, fp32) ----------
            e_sb = big.tile([1, BT], F32, tag="e_sb")
            nc.scalar.activation(e_sb[:], e_ps[:], mybir.ActivationFunctionType.Copy)
            e3 = e_sb[:].rearrange("p (b t) -> p b t", b=BL)
            mx = big.tile([1, BL], F32, tag="mx")
            nc.vector.reduce_max(mx[:], e3, axis=mybir.AxisListType.X)
            ec = big.tile([1, BT], F32, tag="ec")
            nc.vector.tensor_sub(
                ec[:].rearrange("p (b t) -> p b t", b=BL),
                e3,
                mx[:].unsqueeze(2).broadcast_to([1, BL, T]),
            )
            p_sb = big.tile([1, BT], F32, tag="p_sb")
            nc.scalar.activation(p_sb[:], ec[:], mybir.ActivationFunctionType.Exp)
            p3 = p_sb[:].rearrange("p (b t) -> p b t", b=BL)
            s_sb = big.tile([1, BL], F32, tag="s_sb")
            nc.vector.reduce_sum(s_sb[:], p3, axis=mybir.AxisListType.X)
            rs = big.tile([1, BL], F32, tag="rs")
            nc.vector.reciprocal(rs[:], s_sb[:])
            a_sb = big.tile([1, BT], F32, tag="a_sb")
            nc.vector.tensor_mul(
                a_sb[:].rearrange("p (b t) -> p b t", b=BL),
                p3,
                rs[:].unsqueeze(2).broadcast_to([1, BL, T]),
            )
            a_bc = big.tile([128, BT], F32, tag="a_bc")
            nc.gpsimd.partition_broadcast(a_bc[:], a_sb[:])

            # ---------- context^T via DVE (fp32), cast to bf16 ----------
            ctx_sb = big.tile([128, FO, BL], F32, tag="ctx_sb")
            for fo in range(FO):
                prod = work.tile([128, BT], F32, tag="prod")
                nc.vector.tensor_mul(prod[:], padT_sb[:, fo, :], a_bc[:])
                nc.vector.tensor_reduce(
                    ctx_sb[:, fo, :],
                    prod[:].rearrange("p (b t) -> p b t", b=BL),
                    op=mybir.AluOpType.add,
                    axis=mybir.AxisListType.X,
                )
            ctb_sb = big.tile([128, FO, BL], BF16, tag="ctb_sb")
            nc.vector.tensor_copy(ctb_sb[:], ctx_sb[:])

            nc.sync.dma_start(
                ct_in.ap().rearrange("(fo p) b -> p fo b", p=128), ctb_sb[:]
            )
            nc.gpsimd.collective_compute(
                "AllGather",
                mybir.AluOpType.bypass,
                replica_groups=RG,
                ins=[ct_in.ap()],
                outs=[ct_ag.ap()],
            )
            ctT_sb = big.tile([128, FO, NC, BL], BF16, tag="ctT_sb")
            ct_ag_r = ct_ag.ap().rearrange("(j fo p) b -> fo p j b", j=NC, fo=FO, p=128)
            for fo in range(FO):
                nc.sync.dma_start(ctT_sb[:, fo, :, :], ct_ag_r[fo])

            # ---------- 4-layer LSTM cell (i, g, o gates only) ----------
            rhs_sb = ctT_sb  # [128, kt, ...] free size 64 per kt
            for l in range(4):
                kt_n = FO if l == 0 else KT1
                gates = []
                for m in range(3):
                    ps_g = pg.tile([128, B], F32, tag="gates")
                    for kt in range(kt_n):
                        if l == 0:
                            rhs = rhs_sb[:, kt, :, :]
                        else:
                            rhs = rhs_sb[:, kt, :]
                        nc.tensor.matmul(
                            ps_g[:],
                            wl_sb[l][:, kt, m * 128 : (m + 1) * 128],
                            rhs,
                            start=(kt == 0),
                            stop=(kt == kt_n - 1),
                        )
                    gates.append(ps_g)
                sig_i = work.tile([128, B], F32, tag="sig_i")
                nc.scalar.activation(
                    sig_i[:], gates[0][:], mybir.ActivationFunctionType.Sigmoid,
                    bias=b_sb[l][:, 0:1],
                )
                tanh_g = work.tile([128, B], F32, tag="tanh_g")
                nc.scalar.activation(
                    tanh_g[:], gates[1][:], mybir.ActivationFunctionType.Tanh,
                    bias=b_sb[l][:, 1:2],
                )
                c_t = work.tile([128, B], F32, tag="c_t")
                nc.vector.tensor_mul(c_t[:], sig_i[:], tanh_g[:])
                tanh_c = work.tile([128, B], F32, tag="tanh_c")
                nc.scalar.activation(
                    tanh_c[:], c_t[:], mybir.ActivationFunctionType.Tanh
                )
                sig_o = work.tile([128, B], F32, tag="sig_o")
                nc.scalar.activation(
                    sig_o[:], gates[2][:], mybir.ActivationFunctionType.Sigmoid,
                    bias=b_sb[l][:, 2:3],
                )
                h_sl = work.tile([128, B], F32, tag="h_sl")
                nc.vector.tensor_mul(h_sl[:], sig_o[:], tanh_c[:])

                if l < 3:
                    h_slb = work.tile([128, B], BF16, tag="h_slb")
                    nc.vector.tensor_copy(h_slb[:], h_sl[:])
                    nc.sync.dma_start(h_in[l].ap(), h_slb[:])
                    nc.gpsimd.collective_compute(
                        "AllGather",
                        mybir.AluOpType.bypass,
                        replica_groups=RG,
                        ins=[h_in[l].ap()],
                        outs=[h_ag[l].ap()],
                    )
                    nxt = big.tile([128, KT1, B], BF16, tag=f"h{l}_sb")
                    nc.sync.dma_start(
                        nxt[:], h_ag[l].ap().rearrange("(kt p) b -> p kt b", p=128)
                    )
                    rhs_sb = nxt
                else:
                    nc.sync.dma_start(out.ap(), h_sl[:])

    nc.compile()
    return nc


def _prep_inputs(padded, align_ws, w_ih, b_ih, b_hh):
    """Build the 8 per-core input maps (host-side sharding / layout only)."""
    padded = np.asarray(padded, dtype=np.float32)
    w1 = np.asarray(align_ws[0], dtype=np.float32)
    shared = {
        "w1p": np.ascontiguousarray(w1[:, H:]).astype(NPBF),      # [256, 2048]
        "w2": np.ascontiguousarray(np.asarray(align_ws[1], dtype=np.float32)).astype(NPBF),
        "w3": np.ascontiguousarray(np.asarray(align_ws[2], dtype=np.float32)).astype(NPBF),
        "w4t": np.asarray(align_ws[3], dtype=np.float32).reshape(AH, 1).astype(NPBF),
    }
    in_maps = []
    for k in range(NC):
        m = dict(shared)
        # padded^T for batches [8k, 8k+8): [2048, 8, 50] -> [2048, 400]
        sl = padded[:, k * BL : (k + 1) * BL, :]          # [50, 8, 2048]
        pT = np.ascontiguousarray(sl.transpose(2, 1, 0)).reshape(HH, BT)
        m["padT"] = pT
        m["padTb"] = pT.astype(NPBF)
        for l in range(4):
            w = np.asarray(w_ih[l], dtype=np.float32)
            rows = np.concatenate(
                [w[g * H + k * 128 : g * H + (k + 1) * 128, :] for g in (0, 2, 3)],
                axis=0,
            )                                              # [384, fin] i,g,o rows
            m[f"wl{l}"] = np.ascontiguousarray(rows.T).astype(NPBF)   # [fin, 384]
            for name, b in (("bih", b_ih[l]), ("bhh", b_hh[l])):
                bb = np.asarray(b, dtype=np.float32)
                bsel = np.stack(
                    [bb[g * H + k * 128 : g * H + (k + 1) * 128] for g in (0, 2, 3)],
                    axis=1,
                )                                          # [128, 3]
                m[f"{name}{l}"] = np.ascontiguousarray(bsel)
        in_maps.append(m)
    return in_maps


def kernel(padded, align_ws, align_bs, w_ih, w_hh, b_ih, b_hh):
    # align_bs and w_hh are mathematically inert: the MLP biases add a
    # t-invariant constant to e (cancels in softmax); w_hh multiplies the
    # zero initial LSTM state.
    if "nc" not in _CACHE:
        _CACHE["nc"] = _build()
    nc = _CACHE["nc"]
    in_maps = _prep_inputs(padded, align_ws, w_ih, b_ih, b_hh)
    res = run_bass_kernel_spmd(nc, in_maps, list(range(NC)))
    hT = np.concatenate([res.results[k]["out"] for k in range(NC)], axis=0)  # [1024, 64]
    h = hT.T                                                                  # [64, 1024]
    return np.ascontiguousarray(
        np.broadcast_to(h[None, :, :], (T, B, H))
    ).astype(np.float32)


# revision 5
# speedup vs baseline: 1.2799x; 1.0848x over previous
"""Trainium2 Bass kernel for nn_DecoderModel_42228118454332.

Key algebraic structure of the reference model:
  - The 4-layer alignment MLP has no nonlinearities, so it composes into a
    single affine map e = x . m + c with m = W1^T W2^T W3^T W4^T.
  - x = [S | padded]; the S-dependent part of e is constant over encoder
    positions t, so it cancels inside softmax(axis=t). Attention weights
    therefore do not depend on the decoder state S at all.
  - The LSTM starts from zero state each step (w_hh sees h0=c0=0) and its
    input (the context) is step-invariant, so the output h is identical for
    all 50 decoder steps. The f-gate multiplies c0=0 and is never used.

Device computation per core k (SPMD over 8 cores):
  - compose m_P = W1P^T @ (W2^T @ (W3^T @ W4^T)) on PE (only the `padded`
    1024:3072 slice of the input features matters).
  - batch shard: core k owns batches [8k, 8k+8). padTb = padded^T bf16
    [2048, 400] feat-major. e = m_P . padTb on PE -> [1, (b,t)]; softmax
    over t in fp32 on partition 0; broadcast a (bf16) to 128 partitions;
    context^T via DVE multiply (bf16) + reduce (fp32 accum), in two halves.
  - AllGather each context half (bf16, [128, 64] dump layout) so the first
    half's L0 matmuls overlap the second half's DVE + collective.
  - LSTM tensor-sharded over the hidden dim: core k owns h rows
    [128k, 128(k+1)) of every layer, i.e. the matching i/g/o gate rows.
    gates = W_sel^T.T @ ct^T on PE (bf16 in, fp32 accum); sigmoid/tanh on
    ACT in fp32; AllGather h^T (bf16) between layers. Final layer writes
    the fp32 h^T slice out.
Host: concat slices -> h^T [1024, 64] -> h [64, 1024] -> broadcast to
  [50, 64, 1024].

Matmul operands are bf16: fp32 matmuls on TRN2 lower to HI/LO pairs (2x
instructions, ~4x PE time) and forgo fast weight load.
"""

import sys

for _p in ("/opt/trn_rl_repo", "/root/.axon_site/_ro/trn_rl_repo"):
    if _p not in sys.path:
        sys.path.insert(0, _p)

import ml_dtypes
import numpy as np

from concourse import bacc, mybir, tile
from concourse.bass_utils import run_bass_kernel_spmd

H = 1024          # hidden size
HH = 2 * H        # encoder feature size
T = 50            # encoder length == decoder steps
B = 64            # batch
AH = 256          # alignment hidden
NC = 8            # cores
BL = B // NC      # batches per core (8)
BT = BL * T       # 400
FO = HH // 128    # 16 feature chunks of padded part
FH = FO // 2      # 8 chunks per context half
KT1 = H // 128    # 8 k-tiles for layers 1..3

F32 = mybir.dt.float32
BF16 = mybir.dt.bfloat16
NPBF = ml_dtypes.bfloat16
RG = [list(range(NC))]

_CACHE = {}


def _build():
    nc = bacc.Bacc("TRN2", target_bir_lowering=False, debug=False, num_devices=NC)

    # ---- kernel I/O ----
    padTb = nc.dram_tensor("padTb", [HH, BT], BF16, kind="ExternalInput")
    w1p = nc.dram_tensor("w1p", [AH, HH], BF16, kind="ExternalInput")
    w2 = nc.dram_tensor("w2", [AH, AH], BF16, kind="ExternalInput")
    w3 = nc.dram_tensor("w3", [AH, AH], BF16, kind="ExternalInput")
    w4t = nc.dram_tensor("w4t", [AH, 1], BF16, kind="ExternalInput")
    wl = [
        nc.dram_tensor(f"wl{l}", [HH if l == 0 else H, 384], BF16, kind="ExternalInput")
        for l in range(4)
    ]
    bih = [nc.dram_tensor(f"bih{l}", [128, 3], F32, kind="ExternalInput") for l in range(4)]
    bhh = [nc.dram_tensor(f"bhh{l}", [128, 3], F32, kind="ExternalInput") for l in range(4)]
    out = nc.dram_tensor("out", [128, B], F32, kind="ExternalOutput")

    # ---- collective internals (bf16) ----
    # ct halves are contributed in SBUF dump order [128, fo*8+b]; rank j's
    # block in the AG output holds feat rows 128j+p.
    ct_in = [nc.dram_tensor(f"ct_in{h}", [128, FH * BL], BF16) for h in range(2)]
    ct_ag = [
        nc.dram_tensor(f"ct_ag{h}", [128 * NC, FH * BL], BF16, addr_space="Shared")
        for h in range(2)
    ]
    h_in = [nc.dram_tensor(f"h_in{l}", [128, B], BF16) for l in range(3)]
    h_ag = [nc.dram_tensor(f"h_ag{l}", [H, B], BF16, addr_space="Shared") for l in range(3)]

    with tile.TileContext(nc) as tc:
        with (
            tc.tile_pool(name="big", bufs=1) as big,
            tc.tile_pool(name="work", bufs=3) as work,
            tc.tile_pool(name="psum", bufs=1, space="PSUM") as pp,
            tc.tile_pool(name="psg", bufs=4, space="PSUM") as pg,
        ):
            # ---------- DMA loads ----------
            # critical path (sync ring): small align weights, then padTb
            w4t_sb = big.tile([128, 2, 1], BF16, tag="w4t_sb")
            nc.sync.dma_start(w4t_sb[:], w4t.ap().rearrange("(kt p) o -> p kt o", p=128))
            w3_sb = big.tile([128, 2, AH], BF16, tag="w3_sb")
            nc.sync.dma_start(w3_sb[:], w3.ap().rearrange("(kt p) j -> p kt j", p=128))
            w2_sb = big.tile([128, 2, AH], BF16, tag="w2_sb")
            nc.sync.dma_start(w2_sb[:], w2.ap().rearrange("(kt p) j -> p kt j", p=128))
            w1p_sb = big.tile([128, 2, HH], BF16, tag="w1p_sb")
            nc.sync.dma_start(w1p_sb[:], w1p.ap().rearrange("(kt p) j -> p kt j", p=128))

            padTb_sb = big.tile([128, FO, BT], BF16, tag="padTb_sb")
            N_PCHUNK = 4
            for c in range(N_PCHUNK):
                fo0 = c * (FO // N_PCHUNK)
                fo1 = (c + 1) * (FO // N_PCHUNK)
                nc.sync.dma_start(
                    padTb_sb[:, fo0:fo1, :],
                    padTb.ap()[fo0 * 128 : fo1 * 128, :].rearrange(
                        "(fo p) bt -> p fo bt", p=128
                    ),
                )

            # weights on the scalar HWDGE ring (off the critical path)
            wl_sb = []
            for l in range(4):
                kt = FO if l == 0 else KT1
                t_ = big.tile([128, kt, 384], BF16, tag=f"wl{l}_sb")
                nc.scalar.dma_start(
                    t_[:], wl[l].ap().rearrange("(kt p) j -> p kt j", p=128)
                )
                wl_sb.append(t_)

            b_sb = []
            for l in range(4):
                t_i = work.tile([128, 3], F32, tag=f"bih{l}")
                nc.scalar.dma_start(t_i[:], bih[l].ap())
                t_h = work.tile([128, 3], F32, tag=f"bhh{l}")
                nc.scalar.dma_start(t_h[:], bhh[l].ap())
                t_b = big.tile([128, 3], F32, tag=f"b{l}")
                nc.vector.tensor_add(t_b[:], t_i[:], t_h[:])
                b_sb.append(t_b)

            # ---------- compose m_P on PE (bf16 in, fp32 accum) ----------
            ps_u = pp.tile([128, 2], F32, tag="ps_u")
            for mt in range(2):
                for kt in range(2):
                    nc.tensor.matmul(
                        ps_u[:, mt : mt + 1],
                        w3_sb[:, kt, mt * 128 : (mt + 1) * 128],
                        w4t_sb[:, kt, :],
                        start=(kt == 0),
                        stop=(kt == 1),
                    )
            u1_sb = big.tile([128, 2], BF16, tag="u1_sb")
            nc.vector.tensor_copy(u1_sb[:], ps_u[:])

            ps_u2 = pp.tile([128, 2], F32, tag="ps_u")
            for mt in range(2):
                for kt in range(2):
                    nc.tensor.matmul(
                        ps_u2[:, mt : mt + 1],
                        w2_sb[:, kt, mt * 128 : (mt + 1) * 128],
                        u1_sb[:, kt : kt + 1],
                        start=(kt == 0),
                        stop=(kt == 1),
                    )
            u2_sb = big.tile([128, 2], BF16, tag="u2_sb")
            nc.vector.tensor_copy(u2_sb[:], ps_u2[:])

            ps_m = pp.tile([128, FO], F32, tag="ps_m")
            for mt in range(FO):
                for kt in range(2):
                    nc.tensor.matmul(
                        ps_m[:, mt : mt + 1],
                        w1p_sb[:, kt, mt * 128 : (mt + 1) * 128],
                        u2_sb[:, kt : kt + 1],
                        start=(kt == 0),
                        stop=(kt == 1),
                    )
            m_sb = big.tile([128, FO], BF16, tag="m_sb")
            nc.vector.tensor_copy(m_sb[:], ps_m[:])

            # ---------- e = m_P . padTb  -> [1, (b, t)] ----------
            e_ps = pp.tile([1, BT], F32, tag="e_ps")
            for fo in range(FO):
                nc.tensor.matmul(
                    e_ps[:],
                    m_sb[:, fo : fo + 1],
                    padTb_sb[:, fo, :],
                    start=(fo == 0),
                    stop=(fo == FO - 1),
                )

            # ---------- softmax over t (partition 0, fp32) ----------
            e_sb = big.tile([1, BT], F32, tag="e_sb")
            nc.scalar.activation(e_sb[:], e_ps[:], mybir.ActivationFunctionType.Copy)
            e3 = e_sb[:].rearrange("p (b t) -> p b t", b=BL)
            mx = big.tile([1, BL], F32, tag="mx")
            nc.vector.reduce_max(mx[:], e3, axis=mybir.AxisListType.X)
            ec = big.tile([1, BT], F32, tag="ec")
            nc.vector.tensor_sub(
                ec[:].rearrange("p (b t) -> p b t", b=BL),
                e3,
                mx[:].unsqueeze(2).broadcast_to([1, BL, T]),
            )
            p_sb = big.tile([1, BT], F32, tag="p_sb")
            nc.scalar.activation(p_sb[:], ec[:], mybir.ActivationFunctionType.Exp)
            p3 = p_sb[:].rearrange("p (b t) -> p b t", b=BL)
            s_sb = big.tile([1, BL], F32, tag="s_sb")
            nc.vector.reduce_sum(s_sb[:], p3, axis=mybir.AxisListType.X)
            rs = big.tile([1, BL], F32, tag="rs")
            nc.vector.reciprocal(rs[:], s_sb[:])
            a_16 = big.tile([1, BT], BF16, tag="a_16")
            nc.vector.tensor_mul(
                a_16[:].rearrange("p (b t) -> p b t", b=BL),
                p3,
                rs[:].unsqueeze(2).broadcast_to([1, BL, T]),
            )
            a_bc = big.tile([128, BT], BF16, tag="a_bc")
            nc.gpsimd.partition_broadcast(a_bc[:], a_16[:])

            # ---------- context^T via DVE, two halves ----------
            # prod bf16 (DVE 2x mode), reduce accumulates fp32, cast to bf16
            ctb = []
            for hh in range(2):
                ctx_sb = work.tile([128, FH, BL], F32, tag="ctx_sb")
                for fi in range(FH):
                    fo = hh * FH + fi
                    prod = work.tile([128, BT], BF16, tag="prod")
                    nc.vector.tensor_mul(prod[:], padTb_sb[:, fo, :], a_bc[:])
                    nc.vector.tensor_reduce(
                        ctx_sb[:, fi, :],
                        prod[:].rearrange("p (b t) -> p b t", b=BL),
                        op=mybir.AluOpType.add,
                        axis=mybir.AxisListType.X,
                    )
                ctb_sb = work.tile([128, FH, BL], BF16, tag="ctb_sb")
                nc.vector.tensor_copy(ctb_sb[:], ctx_sb[:])
                nc.sync.dma_start(ct_in[hh].ap(), ctb_sb[:].rearrange("p a b -> p (a b)"))
                nc.gpsimd.collective_compute(
                    "AllGather",
                    mybir.AluOpType.bypass,
                    replica_groups=RG,
                    ins=[ct_in[hh].ap()],
                    outs=[ct_ag[hh].ap()],
                )
                ctb.append(ctb_sb)

            # gather full ct^T [128, fo, (j, b)] from the two AG outputs
            ctT_sb = big.tile([128, FO, NC, BL], BF16, tag="ctT_sb")
            for hh in range(2):
                ag_r = ct_ag[hh].ap().rearrange(
                    "(j p) (fo b) -> fo p j b", j=NC, p=128, fo=FH, b=BL
                )
                for fi in range(FH):
                    nc.sync.dma_start(ctT_sb[:, hh * FH + fi, :, :], ag_r[fi])

            # ---------- 4-layer LSTM cell (i, g, o gates only) ----------
            rhs_sb = ctT_sb  # [128, kt, ...] free size 64 per kt
            for l in range(4):
                kt_n = FO if l == 0 else KT1
                gates = []
                for m in range(3):
                    ps_g = pg.tile([128, B], F32, tag="gates")
                    for kt in range(kt_n):
                        if l == 0:
                            rhs = rhs_sb[:, kt, :, :]
                        else:
                            rhs = rhs_sb[:, kt, :]
                        nc.tensor.matmul(
                            ps_g[:],
                            wl_sb[l][:, kt, m * 128 : (m + 1) * 128],
                            rhs,
                            start=(kt == 0),
                            stop=(kt == kt_n - 1),
                        )
                    gates.append(ps_g)
                sig_i = work.tile([128, B], F32, tag="sig_i")
                nc.scalar.activation(
                    sig_i[:], gates[0][:], mybir.ActivationFunctionType.Sigmoid,
                    bias=b_sb[l][:, 0:1],
                )
                tanh_g = work.tile([128, B], F32, tag="tanh_g")
                nc.scalar.activation(
                    tanh_g[:], gates[1][:], mybir.ActivationFunctionType.Tanh,
                    bias=b_sb[l][:, 1:2],
                )
                c_t = work.tile([128, B], F32, tag="c_t")
                nc.vector.tensor_mul(c_t[:], sig_i[:], tanh_g[:])
                tanh_c = work.tile([128, B], F32, tag="tanh_c")
                nc.scalar.activation(
                    tanh_c[:], c_t[:], mybir.ActivationFunctionType.Tanh
                )
                sig_o = work.tile([128, B], F32, tag="sig_o")
                nc.scalar.activation(
                    sig_o[:], gates[2][:], mybir.ActivationFunctionType.Sigmoid,
                    bias=b_sb[l][:, 2:3],
                )
                h_sl = work.tile([128, B], F32, tag="h_sl")
                nc.vector.tensor_mul(h_sl[:], sig_o[:], tanh_c[:])

                if l < 3:
                    h_slb = work.tile([128, B], BF16, tag="h_slb")
                    nc.vector.tensor_copy(h_slb[:], h_sl[:])
                    nc.sync.dma_start(h_in[l].ap(), h_slb[:])
                    nc.gpsimd.collective_compute(
                        "AllGather",
                        mybir.AluOpType.bypass,
                        replica_groups=RG,
                        ins=[h_in[l].ap()],
                        outs=[h_ag[l].ap()],
                    )
                    nxt = big.tile([128, KT1, B], BF16, tag=f"h{l}_sb")
                    nc.sync.dma_start(
                        nxt[:], h_ag[l].ap().rearrange("(kt p) b -> p kt b", p=128)
                    )
                    rhs_sb = nxt
                else:
                    nc.sync.dma_start(out.ap(), h_sl[:])

    nc.compile()
    return nc


def _prep_inputs(padded, align_ws, w_ih, b_ih, b_hh):
    """Build the 8 per-core input maps (host-side sharding / layout only)."""
    padded = np.asarray(padded, dtype=np.float32)
    w1 = np.asarray(align_ws[0], dtype=np.float32)
    shared = {
        "w1p": np.ascontiguousarray(w1[:, H:]).astype(NPBF),      # [256, 2048]
        "w2": np.ascontiguousarray(np.asarray(align_ws[1], dtype=np.float32)).astype(NPBF),
        "w3": np.ascontiguousarray(np.asarray(align_ws[2], dtype=np.float32)).astype(NPBF),
        "w4t": np.asarray(align_ws[3], dtype=np.float32).reshape(AH, 1).astype(NPBF),
    }
    in_maps = []
    for k in range(NC):
        m = dict(shared)
        # padded^T for batches [8k, 8k+8): [2048, 8, 50] -> [2048, 400]
        sl = padded[:, k * BL : (k + 1) * BL, :]          # [50, 8, 2048]
        pT = np.ascontiguousarray(sl.transpose(2, 1, 0)).reshape(HH, BT)
        m["padTb"] = pT.astype(NPBF)
        for l in range(4):
            w = np.asarray(w_ih[l], dtype=np.float32)
            rows = np.concatenate(
                [w[g * H + k * 128 : g * H + (k + 1) * 128, :] for g in (0, 2, 3)],
                axis=0,
            )                                              # [384, fin] i,g,o rows
            m[f"wl{l}"] = np.ascontiguousarray(rows.T).astype(NPBF)   # [fin, 384]
            for name, b in (("bih", b_ih[l]), ("bhh", b_hh[l])):
                bb = np.asarray(b, dtype=np.float32)
                bsel = np.stack(
                    [bb[g * H + k * 128 : g * H + (k + 1) * 128] for g in (0, 2, 3)],
                    axis=1,
                )                                          # [128, 3]
                m[f"{name}{l}"] = np.ascontiguousarray(bsel)
        in_maps.append(m)
    return in_maps


def kernel(padded, align_ws, align_bs, w_ih, w_hh, b_ih, b_hh):
    # align_bs and w_hh are mathematically inert: the MLP biases add a
    # t-invariant constant to e (cancels in softmax); w_hh multiplies the
    # zero initial LSTM state.
    if "nc" not in _CACHE:
        _CACHE["nc"] = _build()
    nc = _CACHE["nc"]
    in_maps = _prep_inputs(padded, align_ws, w_ih, b_ih, b_hh)
    res = run_bass_kernel_spmd(nc, in_maps, list(range(NC)))
    hT = np.concatenate([res.results[k]["out"] for k in range(NC)], axis=0)  # [1024, 64]
    h = hT.T                                                                  # [64, 1024]
    return np.ascontiguousarray(
        np.broadcast_to(h[None, :, :], (T, B, H))
    ).astype(np.float32)
